# revision 6
# baseline (speedup 1.0000x reference)
"""MetaNetImageEncoder Trainium2 kernel.

Data-parallel over batch: 8 samples per NeuronCore x 8 cores.

Per core (D=768, N=196 patches, T=8 tasks, BC=8 samples):
  0. warmup:      dummy matmul stream during input DMA so the PE HAM
                  clock-gate opens before phase 1.
  1. base pass:   A = P @ W1 in fp8 DoubleRow (64*W1); relu+accum with
                  64*b1 bias split across ACT and DVE (pool scale folded
                  downstream). fp8 only perturbs the MetaNet coefficients.
  2. MetaNet:     coefs[t,b] via small bf16 matmul chains.
  3. mixing:      M_b = sum_t c[t,b] dW1[t] via block-diagonal coefficient
                  stationary (fp8 data), PSUM partitions (s', b) so each
                  sample's rows are spread across all 16 SBUF ports;
                  PSUM->SBUF copies emit 64*M in bf16.
  4. final pass:  one 128-partition DMA per sample de-interleaves 64*M_b,
                  DVE/GpSimd add 64*W1; it-outer accumulation over 6 live
                  PSUM banks so matmuls start on the first chunk;
                  relu+accum (64x biased) split ACT/DVE.
  5. layer 2:     vst_t = pooled @ dW2[t] in fp8 DoubleRow; final PSUM
                  chain pooled@W2 + sum_t (c/64)*vst + coefs@db2 + b2.
"""
import numpy as np
import ml_dtypes

import concourse.bass as bass
import concourse.mybir as mybir
import concourse.tile as tile
from concourse.vector_clock import ScopedClock
from concourse.bass_utils import run_bass_kernel_spmd

F32 = mybir.dt.float32
BF16 = mybir.dt.bfloat16
FP8 = mybir.dt.float8e4
RELU = mybir.ActivationFunctionType.Relu
DR = mybir.MatmulPerfMode.DoubleRow
ADD = mybir.AluOpType.add
MULT = mybir.AluOpType.mult
MAX = mybir.AluOpType.max

P = 16
D = 768
T = 8
HM = 192
NPAT = 196          # 14*14 patches
B = 64
NCORES = 8
BC = B // NCORES    # 8 samples per core
NB = BC * NPAT      # 1568
KT = D // 128       # 6 k-tiles (bf16)
KC = D // 256       # 3 k-chains (fp8 DoubleRow)
SC = 64.0           # fp8 / bias scale
CW = 400            # padded chunk width (392 used; 400 for 16B stride)

_PATCHED = False


def _apply_tile_patch():
    """This container's walrus allows only one sem wait per instruction;
    TileContext's exit drain attaches one wait per live semaphore. Split
    them onto standalone single-wait nops."""
    global _PATCHED
    if _PATCHED:
        return
    _PATCHED = True

    def _patched(self, tick_clock, wait_clock):
        carrier = self.nc.sync.nop(nofuse=True, hint="drain_waits")
        wait_clock.add_sem_waits(
            carrier.ins, ScopedClock({None: tick_clock.global_clock})
        )
        si = carrier.ins.sync_info
        waits = list(si.on_wait) if si else []
        if len(waits) > 1:
            carrier.ins.sync_info = mybir.SyncInfo(on_wait=[waits[0]], on_update=[])
            for w in waits[1:]:
                extra = self.nc.sync.nop(nofuse=True, hint="drain_waits")
                extra.ins.sync_info = mybir.SyncInfo(on_wait=[w], on_update=[])
        self.nc.sync.drain()
        self.nc.all_engine_barrier()
        popped = self.nc._tile_sem_poison_stack.pop()
        assert popped is self._sem_poison
        self.nc.clear_and_free_semaphores(list(self.sems.allocated().values()))
        self.nc.all_engine_barrier()

    tile.TileContext._drain_and_barrier = _patched


def _split_multi_waits(nc, max_waits: int = 1):
    """Hoist extra sem waits onto same-engine InstNoOp carriers."""
    for f in nc.m.functions:
        for blk in f.blocks:
            out = []
            for inst in blk.instructions:
                si = inst.sync_info
                if si is not None and len(si.on_wait) > max_waits:
                    waits = list(si.on_wait)
                    for i, w in enumerate(waits[:-max_waits]):
                        out.append(mybir.InstNoOp(
                            name=f"{inst.name}-w{i}",
                            sync_info=mybir.SyncInfo(on_wait=[w], on_update=[]),
                            bass_nofuse=True,
                            engine=inst.engine,
                        ))
                    inst.sync_info = mybir.SyncInfo(
                        on_wait=waits[-max_waits:], on_update=list(si.on_update)
                    )
                out.append(inst)
            blk.instructions = out


def build_kernel():
    nc = bass.Bass(target_bir_lowering=False, trn_type="TRN2")

    din = {}
    def inp(name, shape, dt):
        din[name] = nc.dram_tensor(name, shape, dt, kind="ExternalInput")
        return din[name]

    xtf8 = inp("xtf8", (128, 4, KC, 2, CW), FP8)   # P^T fp8, chunk-contiguous
    xt = inp("xt", (128, KT, NB), BF16)            # P^T bf16 (phase 4 rhs)
    w1f8 = inp("w1f8", (128, KC, 2, D), FP8)       # 64*W1 (phase 1 lhsT)
    w1int64 = inp("w1int64", (128, KT, D), BF16)   # 64*W1 (phase 4 add)
    w2 = inp("w2", (128, KT, D), BF16)
    dw1f8 = inp("dw1f8", (8, 128, KT, D), FP8)     # 64*dW1[t, it*128+s*8+icl, j]
    dw2f8 = inp("dw2f8", (T, 128, KC, 2, D), FP8)
    db1 = inp("db1", (T, D), BF16)
    db2 = inp("db2", (T, D), BF16)
    b1t64 = inp("b1t64", (128, KT), F32)           # 64*b1 [j_local, jt]
    b2t = inp("b2t", (128, KT), F32)
    b2r = inp("b2r", (BC, D), F32)                 # b2 replicated over samples
    mw1 = inp("mw1", (128, KT, HM), BF16)
    mb1t = inp("mb1t", (128, 2), F32)
    mw2 = inp("mw2", (128, 2, T), BF16)
    mb2t = inp("mb2t", (T, 1), F32)
    iexp64 = inp("iexp64", (T, 128), BF16)         # 64*repeat(eye(8),16,axis=1)
    mask16f8 = inp("mask16f8", (128, P), FP8)      # [(t,s), s'] = (s==s')
    i8bf = inp("i8bf", (T, T), BF16)               # eye(8)

    out = nc.dram_tensor("out", (BC, D), F32, kind="ExternalOutput")
    debris = nc.dram_tensor("debris", (128, 2), F32, kind="ExternalOutput")

    with tile.TileContext(nc) as tc:
        with (
            tc.tile_pool(name="big", bufs=1) as big,
            tc.tile_pool(name="sm", bufs=1) as sm,
            tc.tile_pool(name="dwp", bufs=2) as dwp,
            tc.tile_pool(name="dw2p", bufs=8) as dw2p,
            tc.tile_pool(name="mxcp", bufs=3) as mxcp,
            tc.tile_pool(name="scr", bufs=3) as scr,
            tc.tile_pool(name="pst", bufs=2, space="PSUM") as pst,
        ):
            # ---------- persistent loads ----------
            # sync: phase-1 xtf8 chunks, then dW1 stream, later deints.
            xtf8_sb = big.tile([128, 4, KC, 2, CW], FP8, tag="xtf8")
            for ch in range(4):
                nc.sync.dma_start(xtf8_sb[:, ch], xtf8[:, ch])
            # scalar: only w1f8 (engine stays free for ACT work).
            w1f8_sb = big.tile([128, KC, 2, D], FP8, tag="w1f8")
            nc.scalar.dma_start(w1f8_sb[:], w1f8[:])
            # gpsimd: everything else (engine idle early).
            b1t64_sb = sm.tile([128, KT], F32, tag="b1t64")
            nc.gpsimd.dma_start(b1t64_sb[:], b1t64[:])
            mask16f8_sb = sm.tile([128, P], FP8, tag="mask16f8")
            nc.gpsimd.dma_start(mask16f8_sb[:], mask16f8[:])
            mw1_sb = sm.tile([128, KT, HM], BF16, tag="mw1")
            nc.gpsimd.dma_start(mw1_sb[:], mw1[:])
            mb1t_sb = sm.tile([128, 2], F32, tag="mb1t")
            nc.gpsimd.dma_start(mb1t_sb[:], mb1t[:])
            mw2_sb = sm.tile([128, 2, T], BF16, tag="mw2")
            nc.gpsimd.dma_start(mw2_sb[:], mw2[:])
            mb2t_sb = sm.tile([T, 1], F32, tag="mb2t")
            nc.gpsimd.dma_start(mb2t_sb[:], mb2t[:])
            iexp64_sb = sm.tile([T, 128], BF16, tag="iexp64")
            nc.gpsimd.dma_start(iexp64_sb[:], iexp64[:])
            i8bf_sb = sm.tile([T, T], BF16, tag="i8bf")
            nc.gpsimd.dma_start(i8bf_sb[:], i8bf[:])
            db1_sb = sm.tile([T, D], BF16, tag="db1")
            nc.gpsimd.dma_start(db1_sb[:], db1[:])
            b2t_sb = sm.tile([128, KT], F32, tag="b2t")
            nc.gpsimd.dma_start(b2t_sb[:], b2t[:])
            b2r_sb = sm.tile([BC, D], F32, tag="b2r")
            nc.gpsimd.dma_start(b2r_sb[:], b2r[:])
            db2_sb = sm.tile([T, D], BF16, tag="db2")
            nc.gpsimd.dma_start(db2_sb[:], db2[:])
            xt_sb = big.tile([128, KT, NB], BF16, tag="xt")
            nc.gpsimd.dma_start(xt_sb[:], xt[:])
            w1int64_sb = big.tile([128, KT, D], BF16, tag="w1int64")
            nc.gpsimd.dma_start(w1int64_sb[:], w1int64[:])
            w2_sb = big.tile([128, KT, D], BF16, tag="w2")
            nc.gpsimd.dma_start(w2_sb[:], w2[:])

            mxiall = big.tile([128, 8, KT, D], FP8, tag="mxiall")   # 64*M

            poolb = sm.tile([128, KT * BC], F32, tag="poolb")   # 64*196*pooled
            pooln = sm.tile([128, KT * BC], F32, tag="pooln")


            # ---------- phase 0: HAM warmup ----------
            with tc.tile_pool(name="psW", bufs=1, space="PSUM") as psW:
                wtile = scr.tile([128, 392], BF16, tag="wt", bufs=1)
                nc.vector.memset(wtile[:], 0.0)
                paw = psW.tile([128, 392], F32, tag="warm")
                for i in range(14):
                    nc.tensor.matmul(paw[:], wtile[:, 0:128], wtile[:],
                                     start=True, stop=True)
                dummy_ro = sm.tile([128, 1], BF16, tag="dummy_ro")
                nc.scalar.activation(dummy_ro[:], wtile[:, 0:1], RELU)
                warm_sb = sm.tile([128, 1], F32, tag="warm_sb")
                nc.vector.tensor_copy(warm_sb[:], paw[:, 0:1])
                nc.sync.dma_start(debris[:, 0:1], warm_sb[:])

            # ---------- phase 1: base pass (fp8 DoubleRow) ----------
            with tc.tile_pool(name="psA", bufs=6, space="PSUM") as psA:
                for jt in range(KT):
                    pas = []
                    for ch in range(4):      # 4 chunks of 392 = 2 samples
                        pas.append(psA.tile([128, 392], F32, tag="a",
                                            name=f"pa{jt}_{ch}"))
                    for kc in range(KC):
                        for ch in range(4):
                            nc.tensor.matmul(
                                pas[ch][:],
                                w1f8_sb[:, kc, :, jt * 128:(jt + 1) * 128],
                                xtf8_sb[:, ch, kc, :, 0:392],
                                start=(kc == 0), stop=(kc == KC - 1),
                                perf_mode=DR)
                    for ch in range(4):
                        ro392 = scr.tile([128, 392], BF16, tag="ro392")
                        nc.scalar.activation(
                            ro392[:], pas[ch][:], RELU,
                            bias=b1t64_sb[:, jt:jt + 1])
                        nc.vector.tensor_reduce(
                            poolb[:, jt * BC + 2 * ch:jt * BC + 2 * ch + 2],
                            ro392[:].rearrange("p (a b) -> p a b", a=2, b=NPAT),
                            axis=mybir.AxisListType.X, op=ADD)

            # ---------- phase 2: MetaNet ----------
            # PE filler so the HAM clock-gate stays open across the
            # low-matmul MetaNet stretch.
            with tc.tile_pool(name="psW2", bufs=1, space="PSUM") as psW2:
                paw2 = psW2.tile([128, 392], F32, tag="warm2")
                for i in range(12):
                    nc.tensor.matmul(paw2[:], wtile[:, 0:128], wtile[:],
                                     start=True, stop=True)
                warm2_sb = sm.tile([128, 1], F32, tag="warm2_sb")
                nc.vector.tensor_copy(warm2_sb[:], paw2[:, 0:1])
                nc.sync.dma_start(debris[:, 1:2], warm2_sb[:])

            poolb_bf = sm.tile([128, KT * BC], BF16, tag="poolbbf")
            nc.scalar.mul(poolb_bf[:], poolb[:], 1.0 / (NPAT * SC))

            # base2^T[e, b] = W2.T @ pooled + b2  (input to MetaNet)
            base2_bf = sm.tile([128, KT * BC], BF16, tag="base2bf")
            for et in range(KT):
                p2 = pst.tile([128, T], F32, tag="tiny")
                for kt in range(KT):
                    nc.tensor.matmul(
                        p2[:], w2_sb[:, kt, et * 128:(et + 1) * 128],
                        poolb_bf[:, kt * BC:(kt + 1) * BC],
                        start=(kt == 0), stop=(kt == KT - 1))
                nc.vector.tensor_scalar_add(
                    base2_bf[:, et * BC:(et + 1) * BC], p2[:],
                    b2t_sb[:, et:et + 1])

            mh0 = sm.tile([128, T], BF16, tag="mh0")
            mh1 = sm.tile([64, T], BF16, tag="mh1")
            for g, mh_g in ((0, mh0), (1, mh1)):
                cols = 128 if g == 0 else 64
                pm = pst.tile([cols, T], F32, tag="tiny")
                for kt in range(KT):
                    nc.tensor.matmul(
                        pm[:], mw1_sb[:, kt, g * 128:g * 128 + cols],
                        base2_bf[:, kt * BC:(kt + 1) * BC],
                        start=(kt == 0), stop=(kt == KT - 1))
                nc.scalar.activation(mh_g[:], pm[:], RELU,
                                     bias=mb1t_sb[:cols, g:g + 1])

            pc = pst.tile([T, T], F32, tag="tiny")
            nc.tensor.matmul(pc[:], mw2_sb[:, 0, :], mh0[:], start=True, stop=False)
            nc.tensor.matmul(pc[:], mw2_sb[0:64, 1, :], mh1[:], start=False, stop=True)
            coefsT = sm.tile([T, T], F32, tag="coefsT")
            nc.vector.tensor_scalar_add(coefsT[:], pc[:], mb2t_sb[:])
            coefsT_bf = sm.tile([T, T], BF16, tag="coefsTbf")
            nc.vector.tensor_copy(coefsT_bf[:], coefsT[:])

            # coefficient replication [128, 8]: crep64[(t,s), b] = 64*c[t, b]
            pr = pst.tile([128, T], F32, tag="tiny")
            nc.tensor.matmul(pr[:], iexp64_sb[:], coefsT_bf[:], start=True, stop=True)
            crep64 = sm.tile([128, T], F32, tag="crep64")
            nc.vector.tensor_copy(crep64[:], pr[:])

            # block-diagonal mixing stationary, (s', b) column order:
            # cb[(t,s), s'*8+b] = 64*c[t,b] * (s==s')
            cb_sb = sm.tile([128, 128], FP8, tag="cb")
            for b in range(BC):
                nc.vector.tensor_scalar_mul(
                    cb_sb[:, b::8], mask16f8_sb[:], crep64[:, b:b + 1])

            # coefsB[b, t] = c[t, b]; cdiag64_t = diag(coefsB[:, t])/64
            pe2 = pst.tile([T, T], F32, tag="tiny")
            nc.tensor.matmul(pe2[:], coefsT_bf[:], i8bf_sb[:], start=True, stop=True)
            coefsB = sm.tile([T, T], F32, tag="coefsB")
            nc.vector.tensor_copy(coefsB[:], pe2[:])
            cdiag64 = sm.tile([T, T, T], BF16, tag="cdiag64")   # [b', t, b]
            for t in range(T):
                nc.vector.tensor_scalar(
                    cdiag64[:, t, :], i8bf_sb[:], coefsB[:, t:t + 1], 1.0 / SC,
                    op0=MULT, op1=MULT)

            # nb1t64[j_local, jt, b] = 64*(b1 + coefs @ db1)
            nb1t64 = sm.tile([128, KT, BC], F32, tag="nb1t64")
            for jt in range(KT):
                pb = pst.tile([128, T], F32, tag="tiny")
                nc.tensor.matmul(pb[:], db1_sb[:, jt * 128:(jt + 1) * 128],
                                 coefsT_bf[:], start=True, stop=True)
                nc.vector.tensor_scalar(
                    nb1t64[:, jt, :], pb[:], SC, b1t64_sb[:, jt:jt + 1],
                    op0=MULT, op1=ADD)
            negnb1t64 = sm.tile([128, KT, BC], F32, tag="negnb1t64")
            nc.vector.tensor_scalar_mul(negnb1t64[:], nb1t64[:], -1.0)
            nb1tN = sm.tile([128, KT, BC], F32, tag="nb1tN")
            nc.vector.tensor_scalar_mul(nb1tN[:], nb1t64[:], float(NPAT))

            # ---------- phase 3: mixing ----------
            # psM partitions = (s', b); copies emit 64*M in bf16.
            with tc.tile_pool(name="psM", bufs=3, space="PSUM") as psM:
                for icl in range(8):
                    dwt = dwp.tile([128, KT, D], FP8, tag="dw")
                    nc.sync.dma_start(dwt[:], dw1f8[icl])
                    for it in range(KT):
                        pm2 = psM.tile([128, 2, 512], F32, tag="m")  # 2 banks
                        for jh in range(2):
                            nc.tensor.matmul(
                                pm2[:, jh, 0:384], cb_sb[:],
                                dwt[:, it, jh * 384:(jh + 1) * 384],
                                start=True, stop=True)
                        dst = mxiall[:, icl, it, :].rearrange(
                            "p (a b) -> p a b", a=2, b=384)
                        if (icl * KT + it) % 2 == 0:
                            nc.vector.tensor_scalar_mul(
                                dst, pm2[:, :, 0:384], 1.0 / SC)
                        else:
                            nc.scalar.mul(dst, pm2[:, :, 0:384], 1.0 / SC)

            # ---------- phase 4: final per-sample pass ----------
            # mxcb = 64*(W1+M_b); it-outer accumulation over 6 live banks.
            # dw2 prefetch on gpsimd (transfers overlap phase 4).
            dwt2s = []
            for t in range(T):
                dwt2s.append(dw2p.tile([128, KC, 2, D], FP8, tag="dw2",
                                       name=f"dwt2_{t}"))
                nc.sync.dma_start(dwt2s[-1][:], dw2f8[t])
            with tc.tile_pool(name="psF", bufs=6, space="PSUM") as psF:
                for b in range(BC):
                    mxcb = mxcp.tile([128, KT, D], BF16, tag="mxcb")
                    nc.gpsimd.dma_start(mxcb[:], mxiall[b::8, :, :, :])
                    for g in range(3):
                        eng = nc.gpsimd if (g == 2 and b % 2) else nc.vector
                        eng.tensor_tensor(
                            mxcb[:, 2 * g:2 * g + 2, :],
                            mxcb[:, 2 * g:2 * g + 2, :],
                            w1int64_sb[:, 2 * g:2 * g + 2, :], op=ADD)
                    pfs = []
                    for jt in range(KT):
                        pfs.append(psF.tile([128, NPAT], F32, tag="f",
                                            name=f"pf{b}_{jt}"))
                    for it in range(KT):
                        for jt in range(KT):
                            nc.tensor.matmul(
                                pfs[jt][:],
                                mxcb[:, it, jt * 128:(jt + 1) * 128],
                                xt_sb[:, it, b * NPAT:(b + 1) * NPAT],
                                start=(it == 0), stop=(it == KT - 1))
                    for jt in range(KT):
                        rtag = "roA" if jt < 5 else "roV"
                        ro = scr.tile([128, NPAT], BF16, tag=rtag,
                                      name=f"ro{b}_{jt}")
                        pcol = pooln[:, jt * BC + b:jt * BC + b + 1]
                        if jt < 5:
                            nc.scalar.activation(
                                ro[:], pfs[jt][:], RELU,
                                bias=nb1t64[:, jt, b:b + 1],
                                accum_out=pcol)
                        else:
                            nc.vector.tensor_scalar(
                                ro[:], pfs[jt][:], negnb1t64[:, jt, b:b + 1],
                                nb1tN[:, jt, b:b + 1],
                                op0=MAX, op1=ADD, accum_out=pcol)

            # ---------- phase 5: layer 2 ----------
            pooln_bf = sm.tile([128, KT * BC], BF16, tag="poolnbf")
            nc.scalar.mul(pooln_bf[:], pooln[:], 1.0 / (NPAT * SC))
            poolf8 = sm.tile([128, KT, P], FP8, tag="poolf8")
            nc.scalar.mul(
                poolf8[:, :, 0:BC],
                pooln[:].rearrange("p (a b) -> p a b", a=KT, b=BC),
                1.0 / (NPAT * SC))

            vst = sm.tile([BC, T, D], BF16, tag="vst")   # 64*(pooled@dW2)
            with tc.tile_pool(name="psV", bufs=4, space="PSUM") as psV:
                for t in range(T):
                    dwt2 = dwt2s[t]
                    for eh in range(2):
                        pv = psV.tile([8, 384], F32, tag="v")
                        for kc in range(KC):
                            nc.tensor.matmul(
                                pv[:],
                                poolf8[:, 2 * kc:2 * kc + 2, 0:BC],
                                dwt2[:, kc, :, eh * 384:(eh + 1) * 384],
                                start=(kc == 0), stop=(kc == KC - 1),
                                perf_mode=DR)
                        nc.vector.tensor_copy(
                            vst[:, t, eh * 384:(eh + 1) * 384], pv[:])

                out_sb = sm.tile([BC, D], F32, tag="out")
                for eh in range(2):
                    po = psV.tile([8, 384], F32, tag="v")
                    for kt in range(KT):
                        nc.tensor.matmul(
                            po[:], pooln_bf[:, kt * BC:(kt + 1) * BC],
                            w2_sb[:, kt, eh * 384:(eh + 1) * 384],
                            start=(kt == 0), stop=False)
                    for t in range(T):
                        nc.tensor.matmul(po[:], cdiag64[:, t, :],
                                         vst[:, t, eh * 384:(eh + 1) * 384],
                                         start=False, stop=False)
                    nc.tensor.matmul(po[:], coefsT_bf[:],
                                     db2_sb[:, eh * 384:(eh + 1) * 384],
                                     start=False, stop=True)
                    nc.vector.tensor_tensor(
                        out_sb[:, eh * 384:(eh + 1) * 384], po[:],
                        b2r_sb[:, eh * 384:(eh + 1) * 384],
                        op=ADD)
                nc.sync.dma_start(out[:], out_sb[:])

    _split_multi_waits(nc)
    return nc


def prep_inputs(x, W1, b1, W2, b2, dW1, db1, dW2, db2, mw1, mb1, mw2, mb2):
    """Host-side layout prep. Returns per-core in_maps."""
    bf = ml_dtypes.bfloat16
    f8 = ml_dtypes.float8_e4m3
    x = np.asarray(x); W1 = np.asarray(W1); W2 = np.asarray(W2)
    b1 = np.asarray(b1); b2 = np.asarray(b2)
    dW1 = np.asarray(dW1); dW2 = np.asarray(dW2)
    db1 = np.asarray(db1); db2 = np.asarray(db2)
    mw1 = np.asarray(mw1); mb1 = np.asarray(mb1)
    mw2 = np.asarray(mw2); mb2 = np.asarray(mb2)

    # patches^T: [B, D, NPAT]
    pt = x.reshape(B, 3, 14, P, 14, P).transpose(0, 1, 3, 5, 2, 4)
    pt = np.ascontiguousarray(pt).reshape(B, D, NPAT)

    w1f8_c = np.ascontiguousarray(
        (SC * W1).reshape(KC, 2, 128, D).transpose(2, 0, 1, 3)).astype(f8)
    w1int64_c = np.ascontiguousarray(
        (SC * W1).reshape(KT, 128, D).transpose(1, 0, 2)).astype(bf)
    w2_c = np.ascontiguousarray(
        W2.reshape(KT, 128, D).transpose(1, 0, 2)).astype(bf)
    # dw1f8[icl, (t,s), it, j] = 64*dW1[t, it*128 + s*8 + icl, j]
    d = (SC * dW1).reshape(T, KT, P, 8, D)     # [t, it, s, icl, j]
    dw1f8_c = np.ascontiguousarray(
        d.transpose(3, 0, 2, 1, 4).reshape(8, 128, KT, D)).astype(f8)
    dw2f8_c = np.ascontiguousarray(
        (SC * dW2).reshape(T, KC, 2, 128, D).transpose(0, 3, 1, 2, 4)).astype(f8)
    db1_c = db1.astype(bf)
    db2_c = db2.astype(bf)
    b1t64_c = np.ascontiguousarray(
        (SC * b1).reshape(KT, 128).T).astype(np.float32)
    b2t_c = np.ascontiguousarray(b2.reshape(KT, 128).T).astype(np.float32)
    b2r_c = np.tile(b2.astype(np.float32), (BC, 1))
    mw1_c = np.ascontiguousarray(
        mw1.reshape(KT, 128, HM).transpose(1, 0, 2)).astype(bf)
    mb1t_c = np.zeros((128, 2), np.float32)
    mb1t_c[:, 0] = mb1[:128]
    mb1t_c[:64, 1] = mb1[128:]
    mw2_c = np.zeros((128, 2, T), np.float32)
    mw2_c[:, 0, :] = mw2[:128]
    mw2_c[:64, 1, :] = mw2[128:]
    mw2_c = mw2_c.astype(bf)
    mb2t_c = mb2.reshape(T, 1).astype(np.float32)
    iexp64_c = (SC * np.repeat(np.eye(T, dtype=np.float32), P, axis=1)).astype(bf)
    mask16f8_c = np.tile(np.eye(P, dtype=np.float32), (8, 1)).astype(f8)
    i8bf_c = np.eye(T, dtype=np.float32).astype(bf)

    shared = dict(
        w1f8=w1f8_c, w1int64=w1int64_c, w2=w2_c, dw1f8=dw1f8_c, dw2f8=dw2f8_c,
        db1=db1_c, db2=db2_c, b1t64=b1t64_c, b2t=b2t_c, b2r=b2r_c,
        mw1=mw1_c, mb1t=mb1t_c, mw2=mw2_c, mb2t=mb2t_c,
        iexp64=iexp64_c, mask16f8=mask16f8_c, i8bf=i8bf_c,
    )

    in_maps = []
    for c in range(NCORES):
        ptc = pt[c * BC:(c + 1) * BC]                  # [BC, D, NPAT]
        # xt[p, it, (b,n)] = ptc[b, it*128+p, n]
        xt_c = np.ascontiguousarray(
            ptc.reshape(BC, KT, 128, NPAT).transpose(2, 1, 0, 3)
        ).reshape(128, KT, NB).astype(bf)
        # xtf8[p, ch, kc, sub, (bi,n) padded to 400]
        t8 = ptc.reshape(4, 2, KC, 2, 128, NPAT).transpose(4, 0, 2, 3, 1, 5)
        # dims now [128, ch, kc, sub, bi, NPAT]
        xtf8_c = np.zeros((128, 4, KC, 2, CW), np.float32)
        xtf8_c[:, :, :, :, 0:392] = t8.reshape(128, 4, KC, 2, 392)
        xtf8_c = xtf8_c.astype(f8)
        m = dict(shared)
        m["xt"] = xt_c
        m["xtf8"] = xtf8_c
        in_maps.append(m)
    return in_maps


_NC_CACHE = {}


def kernel(**inputs) -> np.ndarray:
    _apply_tile_patch()
    if "nc" not in _NC_CACHE:
        _NC_CACHE["nc"] = build_kernel()
    nc = _NC_CACHE["nc"]
    in_maps = prep_inputs(**inputs)
    res = run_bass_kernel_spmd(nc, in_maps, core_ids=list(range(NCORES)))
    return np.concatenate([r["out"] for r in res.results], axis=0)


# revision 7
# speedup vs baseline: 1.0354x; 1.0354x over previous
"""MetaNetImageEncoder Trainium2 kernel.

Data-parallel over batch: 8 samples per NeuronCore x 8 cores.

Per core (D=768, N=196 patches, T=8 tasks, BC=8 samples):
  0. warmup:      dummy matmul stream during input DMA so the PE HAM
                  clock-gate opens before phase 1.
  1. base pass:   A = P @ W1 in fp8 DoubleRow (64*W1); relu+accum with
                  64*b1 bias split across ACT and DVE (pool scale folded
                  downstream). fp8 only perturbs the MetaNet coefficients.
  2. MetaNet:     coefs[t,b] via small bf16 matmul chains.
  3. mixing:      M_b = sum_t c[t,b] dW1[t] via block-diagonal coefficient
                  stationary (fp8 data), PSUM partitions (s', b) so each
                  sample's rows are spread across all 16 SBUF ports;
                  PSUM->SBUF copies emit 64*M in bf16.
  4. final pass:  one 128-partition DMA per sample de-interleaves 64*M_b,
                  DVE/GpSimd add 64*W1; it-outer accumulation over 6 live
                  PSUM banks so matmuls start on the first chunk;
                  relu+accum (64x biased) split ACT/DVE.
  5. layer 2:     vst_t = pooled @ dW2[t] in fp8 DoubleRow; final PSUM
                  chain pooled@W2 + sum_t (c/64)*vst + coefs@db2 + b2.
"""
import numpy as np
import ml_dtypes

import concourse.bass as bass
import concourse.mybir as mybir
import concourse.tile as tile
from concourse.vector_clock import ScopedClock
from concourse.bass_utils import run_bass_kernel_spmd

F32 = mybir.dt.float32
BF16 = mybir.dt.bfloat16
FP8 = mybir.dt.float8e4
RELU = mybir.ActivationFunctionType.Relu
DR = mybir.MatmulPerfMode.DoubleRow
ADD = mybir.AluOpType.add
MULT = mybir.AluOpType.mult
MAX = mybir.AluOpType.max

P = 16
D = 768
T = 8
HM = 192
NPAT = 196          # 14*14 patches
B = 64
NCORES = 8
BC = B // NCORES    # 8 samples per core
NB = BC * NPAT      # 1568
KT = D // 128       # 6 k-tiles (bf16)
KC = D // 256       # 3 k-chains (fp8 DoubleRow)
SC = 64.0           # fp8 / bias scale
CW = 400            # padded chunk width (392 used; 400 for 16B stride)

_PATCHED = False


def _apply_tile_patch():
    """This container's walrus allows only one sem wait per instruction;
    TileContext's exit drain attaches one wait per live semaphore. Split
    them onto standalone single-wait nops."""
    global _PATCHED
    if _PATCHED:
        return
    _PATCHED = True

    def _patched(self, tick_clock, wait_clock):
        carrier = self.nc.sync.nop(nofuse=True, hint="drain_waits")
        wait_clock.add_sem_waits(
            carrier.ins, ScopedClock({None: tick_clock.global_clock})
        )
        si = carrier.ins.sync_info
        waits = list(si.on_wait) if si else []
        if len(waits) > 1:
            carrier.ins.sync_info = mybir.SyncInfo(on_wait=[waits[0]], on_update=[])
            for w in waits[1:]:
                extra = self.nc.sync.nop(nofuse=True, hint="drain_waits")
                extra.ins.sync_info = mybir.SyncInfo(on_wait=[w], on_update=[])
        self.nc.sync.drain()
        self.nc.all_engine_barrier()
        popped = self.nc._tile_sem_poison_stack.pop()
        assert popped is self._sem_poison
        self.nc.clear_and_free_semaphores(list(self.sems.allocated().values()))
        self.nc.all_engine_barrier()

    tile.TileContext._drain_and_barrier = _patched


def _split_multi_waits(nc, max_waits: int = 1):
    """Hoist extra sem waits onto same-engine InstNoOp carriers."""
    for f in nc.m.functions:
        for blk in f.blocks:
            out = []
            for inst in blk.instructions:
                si = inst.sync_info
                if si is not None and len(si.on_wait) > max_waits:
                    waits = list(si.on_wait)
                    for i, w in enumerate(waits[:-max_waits]):
                        out.append(mybir.InstNoOp(
                            name=f"{inst.name}-w{i}",
                            sync_info=mybir.SyncInfo(on_wait=[w], on_update=[]),
                            bass_nofuse=True,
                            engine=inst.engine,
                        ))
                    inst.sync_info = mybir.SyncInfo(
                        on_wait=waits[-max_waits:], on_update=list(si.on_update)
                    )
                out.append(inst)
            blk.instructions = out


def build_kernel():
    nc = bass.Bass(target_bir_lowering=False, trn_type="TRN2")

    din = {}
    def inp(name, shape, dt):
        din[name] = nc.dram_tensor(name, shape, dt, kind="ExternalInput")
        return din[name]

    xtf8 = inp("xtf8", (128, 4, KC, 2, CW), FP8)   # P^T fp8, chunk-contiguous
    xt = inp("xt", (128, KT, NB), BF16)            # P^T bf16 (phase 4 rhs)
    w1f8 = inp("w1f8", (128, KC, 2, D), FP8)       # 64*W1 (phase 1 lhsT)
    w1int64 = inp("w1int64", (128, KT, D), BF16)   # 64*W1 (phase 4 add)
    w2 = inp("w2", (128, KT, D), BF16)
    dw1f8 = inp("dw1f8", (8, 128, KT, D), FP8)     # 64*dW1[t, it*128+s*8+icl, j]
    dw2f8 = inp("dw2f8", (T, 128, KC, 2, D), FP8)
    db1 = inp("db1", (T, D), BF16)
    db2 = inp("db2", (T, D), BF16)
    b1t64 = inp("b1t64", (128, KT), F32)           # 64*b1 [j_local, jt]
    b2t = inp("b2t", (128, KT), F32)
    b2r = inp("b2r", (BC, D), F32)                 # b2 replicated over samples
    mw1 = inp("mw1", (128, KT, HM), BF16)
    mb1t = inp("mb1t", (128, 2), F32)
    mw2 = inp("mw2", (128, 2, T), BF16)
    mb2t = inp("mb2t", (T, 1), F32)
    iexp64 = inp("iexp64", (T, 128), BF16)         # 64*repeat(eye(8),16,axis=1)
    mask16f8 = inp("mask16f8", (128, P), FP8)      # [(t,s), s'] = (s==s')
    i8bf = inp("i8bf", (T, T), BF16)               # eye(8)

    out = nc.dram_tensor("out", (BC, D), F32, kind="ExternalOutput")
    debris = nc.dram_tensor("debris", (128, 2), F32, kind="ExternalOutput")

    with tile.TileContext(nc) as tc:
        with (
            tc.tile_pool(name="big", bufs=1) as big,
            tc.tile_pool(name="sm", bufs=1) as sm,
            tc.tile_pool(name="dwp", bufs=2) as dwp,
            tc.tile_pool(name="dw2p", bufs=8) as dw2p,
            tc.tile_pool(name="mxcp", bufs=3) as mxcp,
            tc.tile_pool(name="scr", bufs=3) as scr,
            tc.tile_pool(name="pst", bufs=2, space="PSUM") as pst,
        ):
            # ---------- persistent loads ----------
            # sync: phase-1 xtf8 chunks, then dW1 stream, later deints.
            xtf8_sb = big.tile([128, 4, KC, 2, CW], FP8, tag="xtf8")
            for ch in range(4):
                nc.sync.dma_start(xtf8_sb[:, ch], xtf8[:, ch])
            # scalar: only w1f8 (engine stays free for ACT work).
            w1f8_sb = big.tile([128, KC, 2, D], FP8, tag="w1f8")
            nc.scalar.dma_start(w1f8_sb[:], w1f8[:])
            # gpsimd: everything else (engine idle early).
            b1t64_sb = sm.tile([128, KT], F32, tag="b1t64")
            nc.gpsimd.dma_start(b1t64_sb[:], b1t64[:])
            mask16f8_sb = sm.tile([128, P], FP8, tag="mask16f8")
            nc.gpsimd.dma_start(mask16f8_sb[:], mask16f8[:])
            mw1_sb = sm.tile([128, KT, HM], BF16, tag="mw1")
            nc.gpsimd.dma_start(mw1_sb[:], mw1[:])
            mb1t_sb = sm.tile([128, 2], F32, tag="mb1t")
            nc.gpsimd.dma_start(mb1t_sb[:], mb1t[:])
            mw2_sb = sm.tile([128, 2, T], BF16, tag="mw2")
            nc.gpsimd.dma_start(mw2_sb[:], mw2[:])
            mb2t_sb = sm.tile([T, 1], F32, tag="mb2t")
            nc.gpsimd.dma_start(mb2t_sb[:], mb2t[:])
            iexp64_sb = sm.tile([T, 128], BF16, tag="iexp64")
            nc.gpsimd.dma_start(iexp64_sb[:], iexp64[:])
            i8bf_sb = sm.tile([T, T], BF16, tag="i8bf")
            nc.gpsimd.dma_start(i8bf_sb[:], i8bf[:])
            db1_sb = sm.tile([T, D], BF16, tag="db1")
            nc.gpsimd.dma_start(db1_sb[:], db1[:])
            b2t_sb = sm.tile([128, KT], F32, tag="b2t")
            nc.gpsimd.dma_start(b2t_sb[:], b2t[:])
            b2r_sb = sm.tile([BC, D], F32, tag="b2r")
            nc.gpsimd.dma_start(b2r_sb[:], b2r[:])
            db2_sb = sm.tile([T, D], BF16, tag="db2")
            nc.gpsimd.dma_start(db2_sb[:], db2[:])
            xt_sb = big.tile([128, KT, NB], BF16, tag="xt")
            nc.gpsimd.dma_start(xt_sb[:], xt[:])
            w1int64_sb = big.tile([128, KT, D], BF16, tag="w1int64")
            nc.gpsimd.dma_start(w1int64_sb[:], w1int64[:])
            w2_sb = big.tile([128, KT, D], BF16, tag="w2")
            nc.gpsimd.dma_start(w2_sb[:], w2[:])

            mxiall = big.tile([128, 8, KT, D], FP8, tag="mxiall")   # 64*M

            poolb = sm.tile([128, KT * BC], F32, tag="poolb")   # 64*196*pooled
            pooln = sm.tile([128, KT * BC], F32, tag="pooln")


            # ---------- phase 0: HAM warmup ----------
            with tc.tile_pool(name="psW", bufs=1, space="PSUM") as psW:
                wtile = scr.tile([128, 392], BF16, tag="wt", bufs=1)
                nc.vector.memset(wtile[:], 0.0)
                paw = psW.tile([128, 392], F32, tag="warm")
                for i in range(14):
                    nc.tensor.matmul(paw[:], wtile[:, 0:128], wtile[:],
                                     start=True, stop=True)
                dummy_ro = sm.tile([128, 1], BF16, tag="dummy_ro")
                nc.scalar.activation(dummy_ro[:], wtile[:, 0:1], RELU)
                warm_sb = sm.tile([128, 1], F32, tag="warm_sb")
                nc.vector.tensor_copy(warm_sb[:], paw[:, 0:1])
                nc.sync.dma_start(debris[:, 0:1], warm_sb[:])

            # ---------- phase 1: base pass (fp8 DoubleRow) ----------
            with tc.tile_pool(name="psA", bufs=6, space="PSUM") as psA:
                for jt in range(KT):
                    pas = []
                    for ch in range(4):      # 4 chunks of 392 = 2 samples
                        pas.append(psA.tile([128, 392], F32, tag="a",
                                            name=f"pa{jt}_{ch}"))
                    for kc in range(KC):
                        for ch in range(4):
                            nc.tensor.matmul(
                                pas[ch][:],
                                w1f8_sb[:, kc, :, jt * 128:(jt + 1) * 128],
                                xtf8_sb[:, ch, kc, :, 0:392],
                                start=(kc == 0), stop=(kc == KC - 1),
                                perf_mode=DR)
                    for ch in range(4):
                        ro392 = scr.tile([128, 392], BF16, tag="ro392")
                        nc.scalar.activation(
                            ro392[:], pas[ch][:], RELU,
                            bias=b1t64_sb[:, jt:jt + 1])
                        nc.vector.tensor_reduce(
                            poolb[:, jt * BC + 2 * ch:jt * BC + 2 * ch + 2],
                            ro392[:].rearrange("p (a b) -> p a b", a=2, b=NPAT),
                            axis=mybir.AxisListType.X, op=ADD)

            # ---------- phase 2: MetaNet ----------
            # PE filler so the HAM clock-gate stays open across the
            # low-matmul MetaNet stretch.
            with tc.tile_pool(name="psW2", bufs=1, space="PSUM") as psW2:
                paw2 = psW2.tile([128, 392], F32, tag="warm2")
                for i in range(12):
                    nc.tensor.matmul(paw2[:], wtile[:, 0:128], wtile[:],
                                     start=True, stop=True)
                warm2_sb = sm.tile([128, 1], F32, tag="warm2_sb")
                nc.vector.tensor_copy(warm2_sb[:], paw2[:, 0:1])
                nc.sync.dma_start(debris[:, 1:2], warm2_sb[:])

            poolb_bf = sm.tile([128, KT * BC], BF16, tag="poolbbf")
            nc.scalar.mul(poolb_bf[:], poolb[:], 1.0 / (NPAT * SC))

            # base2^T[e, b] = W2.T @ pooled + b2  (input to MetaNet)
            base2_bf = sm.tile([128, KT * BC], BF16, tag="base2bf")
            for et in range(KT):
                p2 = pst.tile([128, T], F32, tag="tiny")
                for kt in range(KT):
                    nc.tensor.matmul(
                        p2[:], w2_sb[:, kt, et * 128:(et + 1) * 128],
                        poolb_bf[:, kt * BC:(kt + 1) * BC],
                        start=(kt == 0), stop=(kt == KT - 1))
                nc.vector.tensor_scalar_add(
                    base2_bf[:, et * BC:(et + 1) * BC], p2[:],
                    b2t_sb[:, et:et + 1])

            mh0 = sm.tile([128, T], BF16, tag="mh0")
            mh1 = sm.tile([64, T], BF16, tag="mh1")
            for g, mh_g in ((0, mh0), (1, mh1)):
                cols = 128 if g == 0 else 64
                pm = pst.tile([cols, T], F32, tag="tiny")
                for kt in range(KT):
                    nc.tensor.matmul(
                        pm[:], mw1_sb[:, kt, g * 128:g * 128 + cols],
                        base2_bf[:, kt * BC:(kt + 1) * BC],
                        start=(kt == 0), stop=(kt == KT - 1))
                nc.scalar.activation(mh_g[:], pm[:], RELU,
                                     bias=mb1t_sb[:cols, g:g + 1])

            pc = pst.tile([T, T], F32, tag="tiny")
            nc.tensor.matmul(pc[:], mw2_sb[:, 0, :], mh0[:], start=True, stop=False)
            nc.tensor.matmul(pc[:], mw2_sb[0:64, 1, :], mh1[:], start=False, stop=True)
            coefsT = sm.tile([T, T], F32, tag="coefsT")
            nc.vector.tensor_scalar_add(coefsT[:], pc[:], mb2t_sb[:])
            coefsT_bf = sm.tile([T, T], BF16, tag="coefsTbf")
            nc.vector.tensor_copy(coefsT_bf[:], coefsT[:])

            # coefficient replication [128, 8]: crep64[(t,s), b] = 64*c[t, b]
            pr = pst.tile([128, T], F32, tag="tiny")
            nc.tensor.matmul(pr[:], iexp64_sb[:], coefsT_bf[:], start=True, stop=True)
            crep64 = sm.tile([128, T], F32, tag="crep64")
            nc.vector.tensor_copy(crep64[:], pr[:])

            # block-diagonal mixing stationary, (s', b) column order:
            # cb[(t,s), s'*8+b] = 64*c[t,b] * (s==s')
            cb_sb = sm.tile([128, 128], FP8, tag="cb")
            for b in range(BC):
                nc.vector.tensor_scalar_mul(
                    cb_sb[:, b::8], mask16f8_sb[:], crep64[:, b:b + 1])

            # coefsB[b, t] = c[t, b]; cdiag64_t = diag(coefsB[:, t])/64
            pe2 = pst.tile([T, T], F32, tag="tiny")
            nc.tensor.matmul(pe2[:], coefsT_bf[:], i8bf_sb[:], start=True, stop=True)
            coefsB = sm.tile([T, T], F32, tag="coefsB")
            nc.vector.tensor_copy(coefsB[:], pe2[:])
            cdiag64 = sm.tile([T, T, T], BF16, tag="cdiag64")   # [b', t, b]
            for t in range(T):
                nc.vector.tensor_scalar(
                    cdiag64[:, t, :], i8bf_sb[:], coefsB[:, t:t + 1], 1.0 / SC,
                    op0=MULT, op1=MULT)

            # nb1t64[j_local, jt, b] = 64*(b1 + coefs @ db1)
            nb1t64 = sm.tile([128, KT, BC], F32, tag="nb1t64")
            for jt in range(KT):
                pb = pst.tile([128, T], F32, tag="tiny")
                nc.tensor.matmul(pb[:], db1_sb[:, jt * 128:(jt + 1) * 128],
                                 coefsT_bf[:], start=True, stop=True)
                nc.vector.tensor_scalar(
                    nb1t64[:, jt, :], pb[:], SC, b1t64_sb[:, jt:jt + 1],
                    op0=MULT, op1=ADD)
            negnb1t64 = sm.tile([128, KT, BC], F32, tag="negnb1t64")
            nc.vector.tensor_scalar_mul(negnb1t64[:], nb1t64[:], -1.0)
            nb1tN = sm.tile([128, KT, BC], F32, tag="nb1tN")
            nc.vector.tensor_scalar_mul(nb1tN[:], nb1t64[:], float(NPAT))

            # ---------- phase 3: mixing ----------
            # psM partitions = (s', b); copies emit 64*M in bf16.
            with tc.tile_pool(name="psM", bufs=3, space="PSUM") as psM:
                for icl in range(8):
                    dwt = dwp.tile([128, KT, D], FP8, tag="dw")
                    nc.sync.dma_start(dwt[:], dw1f8[icl])
                    for it in range(KT):
                        pm2 = psM.tile([128, 2, 512], F32, tag="m")  # 2 banks
                        for jh in range(2):
                            nc.tensor.matmul(
                                pm2[:, jh, 0:384], cb_sb[:],
                                dwt[:, it, jh * 384:(jh + 1) * 384],
                                start=True, stop=True)
                        dst = mxiall[:, icl, it, :].rearrange(
                            "p (a b) -> p a b", a=2, b=384)
                        if (icl * KT + it) % 2 == 0:
                            nc.vector.tensor_scalar_mul(
                                dst, pm2[:, :, 0:384], 1.0 / SC)
                        else:
                            nc.scalar.mul(dst, pm2[:, :, 0:384], 1.0 / SC)

            # ---------- phase 4: final per-sample pass ----------
            # mxcb = 64*(W1+M_b); it-outer accumulation over 6 live banks.
            # dw2 prefetch on gpsimd (transfers overlap phase 4).
            dwt2s = []
            for t in range(T):
                dwt2s.append(dw2p.tile([128, KC, 2, D], FP8, tag="dw2",
                                       name=f"dwt2_{t}"))
                nc.sync.dma_start(dwt2s[-1][:], dw2f8[t])
            with tc.tile_pool(name="psF", bufs=6, space="PSUM") as psF:
                for b in range(BC):
                    mxcb = mxcp.tile([128, KT, D], BF16, tag="mxcb")
                    if b < 2:
                        # halved deint: adds/matmuls start at half-transfer
                        nc.gpsimd.dma_start(mxcb[:, 0:4, :],
                                            mxiall[b::8, :, 0:4, :])
                        nc.gpsimd.dma_start(mxcb[:, 4:6, :],
                                            mxiall[b::8, :, 4:6, :])
                    else:
                        nc.gpsimd.dma_start(mxcb[:], mxiall[b::8, :, :, :])
                    for g in range(3):
                        eng = nc.gpsimd if (g == 2 and b % 2) else nc.vector
                        eng.tensor_tensor(
                            mxcb[:, 2 * g:2 * g + 2, :],
                            mxcb[:, 2 * g:2 * g + 2, :],
                            w1int64_sb[:, 2 * g:2 * g + 2, :], op=ADD)
                    pfs = []
                    for jt in range(KT):
                        pfs.append(psF.tile([128, NPAT], F32, tag="f",
                                            name=f"pf{b}_{jt}"))
                    for it in range(KT):
                        for jt in range(KT):
                            nc.tensor.matmul(
                                pfs[jt][:],
                                mxcb[:, it, jt * 128:(jt + 1) * 128],
                                xt_sb[:, it, b * NPAT:(b + 1) * NPAT],
                                start=(it == 0), stop=(it == KT - 1))
                    for jt in range(KT):
                        rtag = "roA" if jt < 5 else "roV"
                        ro = scr.tile([128, NPAT], BF16, tag=rtag,
                                      name=f"ro{b}_{jt}")
                        pcol = pooln[:, jt * BC + b:jt * BC + b + 1]
                        if jt < 5:
                            nc.scalar.activation(
                                ro[:], pfs[jt][:], RELU,
                                bias=nb1t64[:, jt, b:b + 1],
                                accum_out=pcol)
                        else:
                            nc.vector.tensor_scalar(
                                ro[:], pfs[jt][:], negnb1t64[:, jt, b:b + 1],
                                nb1tN[:, jt, b:b + 1],
                                op0=MAX, op1=ADD, accum_out=pcol)

            # ---------- phase 5: layer 2 ----------
            pooln_bf = sm.tile([128, KT * BC], BF16, tag="poolnbf")
            nc.scalar.mul(pooln_bf[:], pooln[:], 1.0 / (NPAT * SC))
            poolf8 = sm.tile([128, KT, P], FP8, tag="poolf8")
            nc.scalar.mul(
                poolf8[:, :, 0:BC],
                pooln[:].rearrange("p (a b) -> p a b", a=KT, b=BC),
                1.0 / (NPAT * SC))

            vst = sm.tile([BC, T, D], BF16, tag="vst")   # 64*(pooled@dW2)
            with tc.tile_pool(name="psV", bufs=4, space="PSUM") as psV:
                for t in range(T):
                    dwt2 = dwt2s[t]
                    for eh in range(2):
                        pv = psV.tile([8, 384], F32, tag="v")
                        for kc in range(KC):
                            nc.tensor.matmul(
                                pv[:],
                                poolf8[:, 2 * kc:2 * kc + 2, 0:BC],
                                dwt2[:, kc, :, eh * 384:(eh + 1) * 384],
                                start=(kc == 0), stop=(kc == KC - 1),
                                perf_mode=DR)
                        nc.vector.tensor_copy(
                            vst[:, t, eh * 384:(eh + 1) * 384], pv[:])

                out_sb = sm.tile([BC, D], F32, tag="out")
                for eh in range(2):
                    po = psV.tile([8, 384], F32, tag="v")
                    for kt in range(KT):
                        nc.tensor.matmul(
                            po[:], pooln_bf[:, kt * BC:(kt + 1) * BC],
                            w2_sb[:, kt, eh * 384:(eh + 1) * 384],
                            start=(kt == 0), stop=False)
                    for t in range(T):
                        nc.tensor.matmul(po[:], cdiag64[:, t, :],
                                         vst[:, t, eh * 384:(eh + 1) * 384],
                                         start=False, stop=False)
                    nc.tensor.matmul(po[:], coefsT_bf[:],
                                     db2_sb[:, eh * 384:(eh + 1) * 384],
                                     start=False, stop=True)
                    nc.vector.tensor_tensor(
                        out_sb[:, eh * 384:(eh + 1) * 384], po[:],
                        b2r_sb[:, eh * 384:(eh + 1) * 384],
                        op=ADD)
                nc.sync.dma_start(out[:], out_sb[:])

    _split_multi_waits(nc)
    return nc


def prep_inputs(x, W1, b1, W2, b2, dW1, db1, dW2, db2, mw1, mb1, mw2, mb2):
    """Host-side layout prep. Returns per-core in_maps."""
    bf = ml_dtypes.bfloat16
    f8 = ml_dtypes.float8_e4m3
    x = np.asarray(x); W1 = np.asarray(W1); W2 = np.asarray(W2)
    b1 = np.asarray(b1); b2 = np.asarray(b2)
    dW1 = np.asarray(dW1); dW2 = np.asarray(dW2)
    db1 = np.asarray(db1); db2 = np.asarray(db2)
    mw1 = np.asarray(mw1); mb1 = np.asarray(mb1)
    mw2 = np.asarray(mw2); mb2 = np.asarray(mb2)

    # patches^T: [B, D, NPAT]
    pt = x.reshape(B, 3, 14, P, 14, P).transpose(0, 1, 3, 5, 2, 4)
    pt = np.ascontiguousarray(pt).reshape(B, D, NPAT)

    w1f8_c = np.ascontiguousarray(
        (SC * W1).reshape(KC, 2, 128, D).transpose(2, 0, 1, 3)).astype(f8)
    w1int64_c = np.ascontiguousarray(
        (SC * W1).reshape(KT, 128, D).transpose(1, 0, 2)).astype(bf)
    w2_c = np.ascontiguousarray(
        W2.reshape(KT, 128, D).transpose(1, 0, 2)).astype(bf)
    # dw1f8[icl, (t,s), it, j] = 64*dW1[t, it*128 + s*8 + icl, j]
    d = (SC * dW1).reshape(T, KT, P, 8, D)     # [t, it, s, icl, j]
    dw1f8_c = np.ascontiguousarray(
        d.transpose(3, 0, 2, 1, 4).reshape(8, 128, KT, D)).astype(f8)
    dw2f8_c = np.ascontiguousarray(
        (SC * dW2).reshape(T, KC, 2, 128, D).transpose(0, 3, 1, 2, 4)).astype(f8)
    db1_c = db1.astype(bf)
    db2_c = db2.astype(bf)
    b1t64_c = np.ascontiguousarray(
        (SC * b1).reshape(KT, 128).T).astype(np.float32)
    b2t_c = np.ascontiguousarray(b2.reshape(KT, 128).T).astype(np.float32)
    b2r_c = np.tile(b2.astype(np.float32), (BC, 1))
    mw1_c = np.ascontiguousarray(
        mw1.reshape(KT, 128, HM).transpose(1, 0, 2)).astype(bf)
    mb1t_c = np.zeros((128, 2), np.float32)
    mb1t_c[:, 0] = mb1[:128]
    mb1t_c[:64, 1] = mb1[128:]
    mw2_c = np.zeros((128, 2, T), np.float32)
    mw2_c[:, 0, :] = mw2[:128]
    mw2_c[:64, 1, :] = mw2[128:]
    mw2_c = mw2_c.astype(bf)
    mb2t_c = mb2.reshape(T, 1).astype(np.float32)
    iexp64_c = (SC * np.repeat(np.eye(T, dtype=np.float32), P, axis=1)).astype(bf)
    mask16f8_c = np.tile(np.eye(P, dtype=np.float32), (8, 1)).astype(f8)
    i8bf_c = np.eye(T, dtype=np.float32).astype(bf)

    shared = dict(
        w1f8=w1f8_c, w1int64=w1int64_c, w2=w2_c, dw1f8=dw1f8_c, dw2f8=dw2f8_c,
        db1=db1_c, db2=db2_c, b1t64=b1t64_c, b2t=b2t_c, b2r=b2r_c,
        mw1=mw1_c, mb1t=mb1t_c, mw2=mw2_c, mb2t=mb2t_c,
        iexp64=iexp64_c, mask16f8=mask16f8_c, i8bf=i8bf_c,
    )

    in_maps = []
    for c in range(NCORES):
        ptc = pt[c * BC:(c + 1) * BC]                  # [BC, D, NPAT]
        # xt[p, it, (b,n)] = ptc[b, it*128+p, n]
        xt_c = np.ascontiguousarray(
            ptc.reshape(BC, KT, 128, NPAT).transpose(2, 1, 0, 3)
        ).reshape(128, KT, NB).astype(bf)
        # xtf8[p, ch, kc, sub, (bi,n) padded to 400]
        t8 = ptc.reshape(4, 2, KC, 2, 128, NPAT).transpose(4, 0, 2, 3, 1, 5)
        # dims now [128, ch, kc, sub, bi, NPAT]
        xtf8_c = np.zeros((128, 4, KC, 2, CW), np.float32)
        xtf8_c[:, :, :, :, 0:392] = t8.reshape(128, 4, KC, 2, 392)
        xtf8_c = xtf8_c.astype(f8)
        m = dict(shared)
        m["xt"] = xt_c
        m["xtf8"] = xtf8_c
        in_maps.append(m)
    return in_maps


_NC_CACHE = {}


def kernel(**inputs) -> np.ndarray:
    _apply_tile_patch()
    if "nc" not in _NC_CACHE:
        _NC_CACHE["nc"] = build_kernel()
    nc = _NC_CACHE["nc"]
    in_maps = prep_inputs(**inputs)
    res = run_bass_kernel_spmd(nc, in_maps, core_ids=list(range(NCORES)))
    return np.concatenate([r["out"] for r in res.results], axis=0)
